# revision 41
# baseline (speedup 1.0000x reference)
"""Fused multi-head self-attention (T=2048, B=2, E=1024, H=16) on 8 TRN2 cores.

Sharding: batch*heads across cores — core c handles b = c//4, heads
[(c%4)*4, (c%4)*4+4). Projections are column-split (Wq/Wk/Wv) per core's
heads; Wo is row-split with the cross-core reduction done on the host
during unshard (4 partial [T,E] sums per batch element).

v2 schedule — built around the TRN2 PE p-state ramp (0.65 -> 1.2 ->
2.4 GHz after 3us of gap-free execution): the tensor engine must never
idle, so all projection / out-projection / normalize work is cut into
~1-matmul "filler units" that are popped between the attention matmuls.

Device kernel (per core, identical SPMD program):
  - xT is DMA'd chunk-by-chunk (8 tiles) and the first Q/K/V projection
    groups are gated per-chunk, so the PE starts ~2us after launch and
    stays busy through the input load (6 concurrent PSUM accum groups
    round-robin through the pss/attno/big tile slots).
  - qT/kT produced transposed [64*2-pair, T] so scores need no
    transposes; the two heads of a pair sit on partitions 0-63 / 64-127,
    so their K=64 score matmuls run CONCURRENTLY on disjoint PE row
    groups.
  - scores computed transposed sT[s,m] = kT.T @ qT; softmax reductions
    avoided: exp on ScalarE straight out of PSUM, denominators via a
    ones-column appended to v (row 64 of the AV accumulation), normalize
    by reciprocal_approx_fast (single custom-DVE op, ~5x faster than
    nc.vector.reciprocal) + K=1 broadcast matmul + DVE multiply.
  - causal structure: fully-masked 128x128 blocks are skipped AND the
    score/exp/AV column ranges are trimmed to the live (trailing) blocks
    of each diagonal s-tile; binary-mask diagonal blocks applied
    post-exp as a 0/1 multiply on GpSimd; general additive blocks added
    pre-exp on VectorE.
  - AV for s-tile i is emitted one tile late (after scores/exp of tile
    i+1), so the exp latency is always covered by real PE work and the
    PE never waits on ScalarE.
  - out-projection PSUM->SBUF copies run on GpSimd, q/k/v copies on
    VectorE; output is written fp16 (host sums the 4 row-split partials
    in fp32).
  - matmuls in fp16 with fp32 PSUM accumulation.
  - one transient-NaN retry; numpy fallback for exotic masks/key padding
"""
import os
import sys

import numpy as np

for _p in ("/opt/trn_rl_repo", "/root/.axon_site/_ro/trn_rl_repo"):
    if os.path.isdir(_p) and _p not in sys.path:
        sys.path.insert(0, _p)
        break

import concourse.bacc as bacc
import concourse.mybir as mybir
import concourse.tile as tile
from concourse.bass_utils import run_bass_kernel_spmd

f32 = mybir.dt.float32
bf16 = mybir.dt.float16
AF = mybir.ActivationFunctionType

T, B, E, H, HD = 2048, 2, 1024, 16, 64
NCORES = 8
HL = (B * H) // NCORES          # heads per core = 4
J = HL * HD                     # per-core projection width = 256
EC = E // 128                   # e-chunks = 8
SCALE = HD ** -0.5
MCH = 512                       # m-chunk width
NEG_THRESH = -1e8               # "fully masked" threshold

SKIP, ZERO, ADD, ADDBIN = 0, 1, 2, 3

_prog_cache = {}


def _classify_mask(mask):
    """Classify 128x128 blocks of mask[t_query, s_key]."""
    nb = mask.shape[0] // 128
    blocks = mask.reshape(nb, 128, nb, 128)
    all_skip = (blocks <= NEG_THRESH).all(axis=(1, 3))
    all_zero = (blocks == 0.0).all(axis=(1, 3))
    binary = ((blocks == 0.0) | (blocks <= NEG_THRESH)).all(axis=(1, 3))
    cls = np.where(all_skip, SKIP,
                   np.where(all_zero, ZERO, np.where(binary, ADDBIN, ADD)))
    return cls  # [m_block, s_block]


def _build(T_, cls_key):
    cls = np.array(cls_key, dtype=np.int64)
    NB = T_ // 128
    NMC = T_ // MCH
    add_blocks = [(mb, sb) for mb in range(NB) for sb in range(NB)
                  if cls[mb, sb] == ADD]
    add_pos = {blk: i for i, blk in enumerate(add_blocks)}
    n_add = len(add_blocks)
    bin_blocks = [(mb, sb) for mb in range(NB) for sb in range(NB)
                  if cls[mb, sb] == ADDBIN]
    bin_pos = {blk: i for i, blk in enumerate(bin_blocks)}
    n_bin = len(bin_blocks)

    nc = bacc.Bacc("TRN2", target_bir_lowering=False, debug=False)
    xT = nc.declare_dram_parameter("xT", [E, T_], bf16, isOutput=False)
    wqpack = nc.declare_dram_parameter("wqpack", [128, EC * J], bf16,
                                       isOutput=False)
    wkvpack = nc.declare_dram_parameter("wkvpack", [128, 2 * EC * J], bf16,
                                        isOutput=False)
    wopack = nc.declare_dram_parameter("wopack", [128, (J // 128) * E], bf16,
                                       isOutput=False)
    bqp = nc.declare_dram_parameter("bqp", [128, 2], f32, isOutput=False)
    ones1 = nc.declare_dram_parameter("ones1", [1, 64], bf16, isOutput=False)
    msk = nc.declare_dram_parameter("msk", [128, max(n_add, 1) * 128], f32,
                                    isOutput=False)
    tri = nc.declare_dram_parameter("tri", [128, max(n_bin, 1) * 128], bf16,
                                    isOutput=False)
    out = nc.declare_dram_parameter("out", [T_, E], bf16, isOutput=True)

    # live (non-skip) trailing range per (chunk n, s-tile i): first live
    # 128-block within the chunk.  For causal masks the live blocks of a
    # chunk are a trailing contiguous range; fall back to ms=0 otherwise.
    def live_start(n, i):
        ks = [k for k in range(4) if cls[n * 4 + k, i] != SKIP]
        if not ks:
            return None
        k0 = min(ks)
        if ks != list(range(k0, 4)):
            return 0  # non-contiguous pattern: compute the full chunk
        return k0 * 128

    with tile.TileContext(nc) as tc:
        with nc.allow_low_precision(reason="bf16 matmuls, fp32 psum"), \
             tc.tile_pool(name="sba", bufs=1) as sba, \
             tc.tile_pool(name="sbw", bufs=1) as sbw, \
             tc.tile_pool(name="ps", bufs=1, space="PSUM") as ps:
            xT_cs = [sba.tile([128, T_], bf16, name=f"xT_c{c}")
                     for c in range(EC)]
            wpack_sb = sba.tile([128, 3 * EC * J], bf16)
            wq_sb = wpack_sb[:, 0:EC * J]
            wk_sb = wpack_sb[:, EC * J:2 * EC * J]
            wv_sb = wpack_sb[:, 2 * EC * J:3 * EC * J]
            wo_sb = sba.tile([128, (J // 128) * E], bf16)
            qT_sb = sba.tile([128, 2 * T_], bf16)
            kT_sb = sba.tile([128, 2 * T_], bf16)
            v_sb = sba.tile([128, HL * NB * 128], bf16)
            oT_sb = sba.tile([128, 2 * T_], bf16)
            bq_sb = sba.tile([128, 2], f32)
            msk_sb = sba.tile([128, max(n_add, 1) * 128], f32)
            tri_sb = sba.tile([128, max(n_bin, 1) * 128], bf16)
            ones1_sb = sba.tile([1, 64], bf16)

            # ---- input DMAs: wq first, then xT chunk-by-chunk so the
            # first projection matmuls start as soon as chunk 0 lands ----
            nc.sync.dma_start(wq_sb, wqpack[:, :])
            nc.sync.dma_start(bq_sb[:], bqp[:, :])
            nc.sync.dma_start(ones1_sb[:], ones1[:, :])
            for c in range(EC):
                nc.sync.dma_start(xT_cs[c][:], xT[c * 128:(c + 1) * 128, :])
            nc.sync.dma_start(wpack_sb[:, EC * J:3 * EC * J], wkvpack[:, :])
            nc.sync.dma_start(wo_sb[:], wopack[:, :])
            # Each v strip is 128 wide: ones at col 0 (denominator lands
            # on PSUM partition 0 where the custom-DVE reciprocal needs
            # it), v at cols 64-127 (o lands on partitions 64-127, a
            # 64-aligned PSUM range for the DVE normalize multiply),
            # zeros in cols 1-63.
            v_view = v_sb[:].rearrange("p (x c) -> p x c", c=128)
            nc.vector.memset(v_view[:, :, 0:1], 1.0)
            nc.vector.memset(v_view[:, :, 1:64], 0.0)
            if n_add:
                nc.sync.dma_start(msk_sb[:], msk[:, :])
            if n_bin:
                nc.sync.dma_start(tri_sb[:], tri[:, :])

            # ================= projection building blocks ==============
            # Each "group" is one PSUM accumulation (8 chunk matmuls) +
            # one copy-out, cut into one-matmul thunks.  The PSUM tile is
            # allocated lazily inside the FIRST thunk so slot-rotation
            # order always matches instruction emission order.
            TB = {"pss": 2, "attno": 3, "big": 1}

            def qk_units(nn, u, wsb, dst, biased, tag):
                cell = []

                def mm(c):
                    def go():
                        if not cell:
                            cell.append(ps.tile([128, 512], f32, tag=tag,
                                                bufs=TB[tag], name="psq"))
                        nc.tensor.matmul(
                            cell[0][:],
                            wsb[:, c * J + u * 128: c * J + (u + 1) * 128],
                            xT_cs[c][:, nn * 512: nn * 512 + 512],
                            start=(c == 0), stop=(c == EC - 1),
                            skip_group_check=True)
                    return go

                def cp():
                    dslc = dst[:, u * T_ + nn * 512: u * T_ + nn * 512 + 512]
                    if biased:
                        nc.vector.tensor_scalar_add(dslc, cell[0][:],
                                                    bq_sb[:, u:u + 1])
                    else:
                        nc.vector.tensor_copy(dslc, cell[0][:])
                return [(mm(c), "mm") for c in range(EC)] + [(cp, "cp")]

            def v_units(i, tag):
                cell = []

                def mm(c):
                    def go():
                        if not cell:
                            cell.append(ps.tile([128, 512], f32, tag=tag,
                                                bufs=TB[tag], name="psv"))
                        nc.tensor.matmul(
                            cell[0][:, 0:J],
                            xT_cs[c][:, i * 128: i * 128 + 128],
                            wv_sb[:, c * J:(c + 1) * J],
                            start=(c == 0), stop=(c == EC - 1),
                            skip_group_check=True)
                    return go

                def cp():
                    dstv = v_sb[:, i * (HL * 128): (i + 1) * (HL * 128)] \
                        .rearrange("p (h c) -> p h c", c=128)[:, :, 64:128]
                    nc.vector.tensor_copy(dstv, cell[0][:, 0:J])
                return [(mm(c), "mm") for c in range(EC)] + [(cp, "cp")]

            # ---- prologue: 6 concurrent accumulation groups round-robin
            # over the (not-yet-used) attention psum slots, chunk-gated.
            V_UP = min(12, NB)
            vtags = ["attno", "big"] + ["pss", "pss",
                                        "attno", "attno", "attno", "big"] * 2
            pro_groups = [qk_units(0, 0, wq_sb, qT_sb, True, "pss"),
                          qk_units(0, 0, wk_sb, kT_sb, False, "pss"),
                          qk_units(0, 1, wq_sb, qT_sb, True, "attno"),
                          qk_units(0, 1, wk_sb, kT_sb, False, "attno")]
            pro_groups += [v_units(i, vtags[i]) for i in range(V_UP)]
            active = [list(g) for g in pro_groups[:6]]
            pend = pro_groups[6:]
            while active:
                nxt = []
                for g in active:
                    for _ in range(2):
                        if g:
                            g.pop(0)[0]()
                    if g:
                        nxt.append(g)
                    elif pend:
                        nxt.append(list(pend.pop(0)))
                active = nxt

            # ---- attention-phase filler units -------------------------
            # Each entry is (need_n, thunk): the unit MUST be emitted
            # before attention pair (need_n, 0) starts (emission-order
            # dependency), and is otherwise popped as PE filler.
            from collections import deque
            fill = deque()
            for nn in range(1, NMC):
                for u in range(2):
                    for wsb, dst, biased in ((wq_sb, qT_sb, True),
                                             (wk_sb, kT_sb, False)):
                        for th, kd in qk_units(nn, u, wsb, dst, biased, "big"):
                            fill.append((nn, th, kd))
                for i in range(4 * nn, min(4 * nn + 4, NB)):
                    if i >= V_UP:
                        for th, kd in v_units(i, "big"):
                            fill.append((nn, th, kd))
            for i in range(max(4 * NMC, V_UP), NB):
                for th, kd in v_units(i, "big"):
                    fill.append((NMC - 1, th, kd))

            def pop_fill(k):
                # pop up to k units; stop after a group-final copy so the
                # next group's first matmul (same PSUM slot) is emitted a
                # tile later, giving the copy time to drain (no PE stall)
                n = 0
                while fill and n < k:
                    _, th, kd = fill.popleft()
                    th()
                    n += 1
                    if kd == "cp":
                        break
                return n

            # ---- out-projection groups (become filler as oT completes)
            def out_proj_group(m16, eh, tag="big"):
                pso = ps.tile([128, 512], f32, tag=tag, bufs=TB[tag],
                              name="pso_op")
                for jc in range(J // 128):
                    nc.tensor.matmul(
                        pso[:],
                        oT_sb[:, jc * T_ + m16 * 128: jc * T_ + m16 * 128 + 128],
                        wo_sb[:, jc * E + eh * 512: jc * E + eh * 512 + 512],
                        start=(jc == 0), stop=(jc == J // 128 - 1),
                        skip_group_check=True)
                ob = sbw.tile([128, 512], bf16, tag="ob", bufs=4)
                if (m16 + eh) % 2 == 1:
                    nc.vector.tensor_copy(ob[:], pso[:])
                else:
                    nc.scalar.copy(ob[:], pso[:])
                nc.sync.dma_start(
                    out[m16 * 128:(m16 + 1) * 128,
                        eh * 512:(eh + 1) * 512], ob[:])

            def out_proj_thunks(n, tail=False, lo=0, hi=None):
                # tail groups rotate over the now-idle pss slots as well,
                # so back-to-back groups don't serialize on one PSUM bank
                def grp(m16, eh, tag):
                    return lambda: out_proj_group(m16, eh, tag)
                tags = ["pss", "big"] if tail else ["big"]
                pairs = [(m16, eh) for m16 in range(n * 4, n * 4 + 4)
                         for eh in range(E // 512)]
                return [grp(m16, eh, tags[k % len(tags)])
                        for k, (m16, eh) in enumerate(pairs)][lo:hi]

            # ---- normalize: fast reciprocal + partition-broadcast ----
            def normalize_units(pn, pu, pA, pB):
                """Units freeing psoA then psoB of pair (pn, pu).  No PE
                work at all: DVE reciprocal -> Pool partition_broadcast
                -> DVE multiply (PSUM x SBUF -> SBUF)."""
                recfs = [sbw.tile([1, 512], f32, tag="recf", bufs=4,
                                  name=f"recf{r}") for r in range(2)]

                def recip_half(pso_, r):
                    def go():
                        nc.vector.reciprocal_approx_fast(
                            recfs[r][0:1, 0:512], pso_[0:1, 0:512])
                    return go

                def norm_half(pso_, r, h):
                    u, poff = h >> 1, (h & 1) * 64

                    def go():
                        rb = sbw.tile([64, 512], f32, tag="rb", bufs=2)
                        nc.gpsimd.partition_broadcast(
                            rb[:], recfs[r][0:1, 0:512], channels=64)
                        nc.vector.tensor_mul(
                            oT_sb[poff:poff + 64,
                                  u * T_ + pn * 512: u * T_ + pn * 512 + 512],
                            pso_[64:128, :], rb[:])
                    return go

                return [recip_half(pA, 0), norm_half(pA, 0, 2 * pu),
                        recip_half(pB, 1), norm_half(pB, 1, 2 * pu + 1)]

            # ================= attention main loop =====================
            def s_loop_pair(n, u, work):
                """work: deque of priority units (normalize/out-proj) to
                interleave; falls back to `fill` units."""
                hA, hB = 2 * u, 2 * u + 1
                stiles = [i for i in range(NB) if live_start(n, i) is not None]
                psoA = ps.tile([128, 512], f32, tag="attno", bufs=3,
                               name="psoA")
                psoB = ps.tile([128, 512], f32, tag="attno", bufs=3,
                               name="psoB")
                qbase = u * T_ + n * 512
                last = len(stiles) - 1
                prev = None  # deferred AV: (i, ms, pt)

                def do_av(i, ms, pt, idx):
                    for pso_, h, off in ((psoA, hA, 0), (psoB, hB, 512)):
                        strip = v_sb[:, (i * HL + h) * 128:
                                     (i * HL + h) * 128 + 128]
                        nc.tensor.matmul(
                            pso_[0:128, ms:512], strip[:, :],
                            pt[:, off + ms:off + 512],
                            start=(idx == 0), stop=(idx == last),
                            skip_group_check=True)

                for idx, i in enumerate(stiles):
                    ms = live_start(n, i)
                    zero_ms = ms if idx == 0 else 0
                    if idx == 0:
                        ms = 0  # first tile zero-fills the full AV range
                    pss = ps.tile([128, 1024], f32, tag="pss", bufs=2,
                                  name="pss")
                    kA = kT_sb[0:64, u * T_ + i * 128: u * T_ + i * 128 + 128]
                    kB = kT_sb[64:128, u * T_ + i * 128: u * T_ + i * 128 + 128]
                    qA = qT_sb[0:64, qbase + ms: qbase + 512]
                    qB = qT_sb[64:128, qbase + ms: qbase + 512]
                    nc.tensor.matmul(pss[:, ms:512], kA, qA,
                                     start=True, stop=True,
                                     skip_group_check=True)
                    nc.tensor.matmul(pss[:, 512 + ms:1024], kB, qB,
                                     start=True, stop=True,
                                     skip_group_check=True)
                    for k in range(4):
                        if cls[n * 4 + k, i] == ADD:
                            pos = add_pos[(n * 4 + k, i)]
                            mblk = msk_sb[:, pos * 128:(pos + 1) * 128]
                            for off in (0, 512):
                                nc.vector.tensor_add(
                                    pss[:, off + k * 128: off + (k + 1) * 128],
                                    pss[:, off + k * 128: off + (k + 1) * 128],
                                    mblk)
                    pt = sbw.tile([128, 1024], bf16, tag="pt", bufs=4)
                    # one exp covering the live columns of both heads
                    pv = pss[:].rearrange("p (h m) -> p h m", m=512)[:, :, ms:512]
                    tv = pt[:].rearrange("p (h m) -> p h m", m=512)[:, :, ms:512]
                    nc.scalar.activation(tv, pv, AF.Exp)
                    if zero_ms:
                        # non-causal first tile: zero leading skip blocks
                        zv = pt[:].rearrange("p (h m) -> p h m",
                                             m=512)[:, :, 0:zero_ms]
                        nc.vector.memset(zv, 0.0)
                    # 0/1 diagonal-block masks on DVE (fp16 2x mode, all
                    # SBUF).  GpSimd must run ONLY partition_broadcast —
                    # mixing Q7 library ops there costs ~7us per reload.
                    for k in range(4):
                        if cls[n * 4 + k, i] == ADDBIN:
                            pos = bin_pos[(n * 4 + k, i)]
                            tblk = tri_sb[:, pos * 128:(pos + 1) * 128]
                            for off in (0, 512):
                                nc.vector.tensor_mul(
                                    pt[:, off + k * 128: off + (k + 1) * 128],
                                    pt[:, off + k * 128: off + (k + 1) * 128],
                                    tblk)
                    # filler between this tile's exp and the previous
                    # tile's AV keeps the PE fed while ScalarE works
                    pop_fill(1)
                    if work:
                        work.popleft()()
                    else:
                        pop_fill(1)
                    if prev is not None:
                        do_av(*prev)
                    prev = (i, ms, pt, idx)
                if work:
                    work.popleft()()
                else:
                    pop_fill(1)
                do_av(*prev)
                while work:
                    work.popleft()()
                    pop_fill(1)
                return psoA, psoB

            prevpair = None
            carry = deque()
            held = []
            for n in range(NMC):
                for u in range(2):
                    # mandatory: emit every fill unit pair (n, u) depends on
                    while fill and fill[0][0] <= n:
                        fill.popleft()[1]()
                    work = deque()
                    if prevpair is not None:
                        pn, pu, pA, pB = prevpair
                        work.extend(normalize_units(pn, pu, pA, pB))
                    work.extend(carry)
                    carry = deque()
                    psoA, psoB = s_loop_pair(n, u, work)
                    if prevpair is not None and pu == 1:
                        if pn == NMC - 2:
                            # hold back half of the second-to-last chunk's
                            # out-projection to overlap the final
                            # normalize chain after the last AV
                            carry.extend(out_proj_thunks(pn, lo=0, hi=4))
                            held = out_proj_thunks(pn, tail=True, lo=4)
                        else:
                            carry.extend(out_proj_thunks(pn))
                    prevpair = (n, u, psoA, psoB)
            pn, pu, pA, pB = prevpair
            tail = deque(normalize_units(pn, pu, pA, pB))
            held = deque(held if NMC >= 2 else [])
            # interleave the last pair's normalize (DVE/Pool) with the
            # held-back out-projection groups (PE work)
            while tail or held or carry:
                if carry:
                    carry.popleft()()
                if tail:
                    tail.popleft()()
                if held:
                    held.popleft()()
            while fill:
                fill.popleft()[1]()
            for w in out_proj_thunks(pn, tail=True):
                w()

    nc.compile()
    return nc


def _get_program(T_, cls):
    key = (T_, tuple(map(tuple, cls.tolist())))
    if key not in _prog_cache:
        _prog_cache[key] = _build(T_, key[1])
    return _prog_cache[key]


def _numpy_ref(query, attn_mask, key_padding_mask, Wq, bq, Wk, bk, Wv, bv,
               Wo, bo):
    """Exact-semantics fallback (mirrors reference.py in numpy)."""
    q = (query @ Wq.T + bq) * SCALE
    k = query @ Wk.T + bk
    v = query @ Wv.T + bv

    def shp(x):
        return x.reshape(T, B * H, HD).transpose(1, 0, 2)

    q, k, v = shp(q), shp(k), shp(v)
    w = np.einsum('bth,bsh->bts', q, k).reshape(B, H, T, T) + attn_mask
    w = np.where(key_padding_mask[:, None, None, :], -np.inf, w)
    w = w - w.max(axis=-1, keepdims=True)
    ew = np.exp(w)
    p = (ew / ew.sum(axis=-1, keepdims=True)).reshape(B * H, T, T)
    o = np.einsum('bts,bsh->bth', p, v.reshape(B * H, T, HD))
    o = o.transpose(1, 0, 2).reshape(T, B, E)
    return (o @ Wo.T + bo).astype(np.float32)


def _prep_inputs(query, attn_mask, Wq, bq, Wk, Wv, Wo, cls):
    """Build the 8 per-core input maps."""
    bf = np.float16
    add_blocks = [(mb, sb) for mb in range(T // 128) for sb in range(T // 128)
                  if cls[mb, sb] == ADD]
    n_add = len(add_blocks)
    if n_add:
        mskp = np.empty((128, n_add * 128), np.float32)
        for i, (mb, sb) in enumerate(add_blocks):
            blk = attn_mask[mb * 128:(mb + 1) * 128, sb * 128:(sb + 1) * 128]
            mskp[:, i * 128:(i + 1) * 128] = np.ascontiguousarray(blk.T)
    else:
        mskp = np.zeros((128, 128), np.float32)
    bin_blocks = [(mb, sb) for mb in range(T // 128) for sb in range(T // 128)
                  if cls[mb, sb] == ADDBIN]
    if bin_blocks:
        trip = np.empty((128, len(bin_blocks) * 128), bf)
        for i, (mb, sb) in enumerate(bin_blocks):
            blk = attn_mask[mb * 128:(mb + 1) * 128, sb * 128:(sb + 1) * 128]
            trip[:, i * 128:(i + 1) * 128] = (blk.T == 0.0).astype(bf)
    else:
        trip = np.zeros((128, 128), bf)
    ones1 = np.ones((1, 64), bf)

    in_maps = []
    for core in range(NCORES):
        b = core // (NCORES // B)
        jsl = slice((core % (NCORES // B)) * J, (core % (NCORES // B)) * J + J)
        EC_, J_ = E // 128, J

        def sb_layout(wT):  # [E, J] -> SBUF [128, EC*J]
            return np.ascontiguousarray(
                wT.reshape(EC_, 128, J_).transpose(1, 0, 2).reshape(128, EC_ * J_))

        xT_c = np.ascontiguousarray(query[:, b, :].T).astype(bf)
        wq_l = sb_layout((Wq[jsl, :] * np.float32(SCALE)).T)
        wk_l = sb_layout(Wk[jsl, :].T)
        wv_l = sb_layout(Wv[jsl, :].T)
        wqpack = np.ascontiguousarray(wq_l).astype(bf)
        wkvpack = np.concatenate([wk_l, wv_l], axis=1).astype(bf)
        woT = Wo[:, jsl].T  # [J, E]
        wopack = np.ascontiguousarray(
            woT.reshape(J_ // 128, 128, E).transpose(1, 0, 2)
            .reshape(128, (J_ // 128) * E)).astype(bf)
        bq_c = np.ascontiguousarray(
            (bq[jsl] * np.float32(SCALE)).reshape(2, 128).T)
        in_maps.append({
            "xT": xT_c, "wqpack": wqpack, "wkvpack": wkvpack,
            "wopack": wopack, "bqp": bq_c, "ones1": ones1, "msk": mskp,
            "tri": trip,
        })
    return in_maps


def _kernel_impl(inputs, trace=False, **run_kwargs):
    query = np.asarray(inputs["query"], np.float32)
    attn_mask = np.asarray(inputs["attn_mask"], np.float32)
    kpm = np.asarray(inputs["key_padding_mask"])
    Wq = np.asarray(inputs["Wq"], np.float32)
    bq = np.asarray(inputs["bq"], np.float32)
    Wk = np.asarray(inputs["Wk"], np.float32)
    bk = np.asarray(inputs["bk"], np.float32)
    Wv = np.asarray(inputs["Wv"], np.float32)
    bv = np.asarray(inputs["bv"], np.float32)
    Wo = np.asarray(inputs["Wo"], np.float32)
    bo = np.asarray(inputs["bo"], np.float32)

    # Fast path requires: no key padding, no fully-masked rows, block-
    # classifiable mask with a modest number of additive blocks, and no
    # bk dependence issue (bk shifts are softmax-invariant, always ok).
    cls = _classify_mask(attn_mask)
    fallback = (
        kpm.any()
        or (attn_mask.max(axis=1) <= NEG_THRESH).any()
        or (cls == ADD).sum() > 24 or (cls == ADDBIN).sum() > 24
        or np.isnan(attn_mask).any()
    )
    if fallback:
        return _numpy_ref(query, attn_mask, kpm, Wq, bq, Wk, bk, Wv, bv,
                          Wo, bo), None

    nc = _get_program(T, cls)
    in_maps = _prep_inputs(query, attn_mask, Wq, bq, Wk, Wv, Wo, cls)
    for attempt in range(3):
        res = run_bass_kernel_spmd(nc, in_maps, core_ids=list(range(NCORES)),
                                   trace=trace, **run_kwargs)
        if all(np.isfinite(r["out"]).all() for r in res.results):
            break
    else:
        return _numpy_ref(query, attn_mask, kpm, Wq, bq, Wk, bk, Wv, bv,
                          Wo, bo), None

    # unshard: sum the 4 row-split partials per batch element (the Wo
    # all-reduce), then add bo and the bv contribution (sum_s p = 1).
    bo_total = bo + Wo @ bv
    out = np.empty((T, B, E), np.float32)
    gsz = NCORES // B
    for b in range(B):
        acc = res.results[b * gsz]["out"].astype(np.float32)
        for c in range(b * gsz + 1, (b + 1) * gsz):
            acc = acc + res.results[c]["out"].astype(np.float32)
        out[:, b, :] = acc + bo_total[None, :]
    return out, res


def kernel(**inputs):
    out, _ = _kernel_impl(inputs, trace=False)
    return out


# revision 44
# speedup vs baseline: 1.1705x; 1.1705x over previous
"""Fused multi-head self-attention (T=2048, B=2, E=1024, H=16) on 8 TRN2 cores.

Sharding: batch*heads across cores — core c handles b = c//4, heads
[(c%4)*4, (c%4)*4+4). Projections are column-split (Wq/Wk/Wv) per core's
heads; Wo is row-split with the cross-core reduction done on the host
during unshard (4 partial [T,E] sums per batch element).

v2 schedule — built around the TRN2 PE p-state ramp (0.65 -> 1.2 ->
2.4 GHz after 3us of gap-free execution): the tensor engine must never
idle, so all projection / out-projection / normalize work is cut into
~1-matmul "filler units" that are popped between the attention matmuls.

Device kernel (per core, identical SPMD program):
  - xT is DMA'd chunk-by-chunk (8 tiles) and the first Q/K/V projection
    groups are gated per-chunk, so the PE starts ~2us after launch and
    stays busy through the input load (6 concurrent PSUM accum groups
    round-robin through the pss/attno/big tile slots).
  - qT/kT produced transposed [64*2-pair, T] so scores need no
    transposes; the two heads of a pair sit on partitions 0-63 / 64-127,
    so their K=64 score matmuls run CONCURRENTLY on disjoint PE row
    groups.
  - scores computed transposed sT[s,m] = kT.T @ qT; softmax reductions
    avoided: exp on ScalarE straight out of PSUM, denominators via a
    ones-column appended to v (row 64 of the AV accumulation), normalize
    by reciprocal_approx_fast (single custom-DVE op, ~5x faster than
    nc.vector.reciprocal) + K=1 broadcast matmul + DVE multiply.
  - causal structure: fully-masked 128x128 blocks are skipped AND the
    score/exp/AV column ranges are trimmed to the live (trailing) blocks
    of each diagonal s-tile; binary-mask diagonal blocks applied
    post-exp as a 0/1 multiply on GpSimd; general additive blocks added
    pre-exp on VectorE.
  - AV for s-tile i is emitted one tile late (after scores/exp of tile
    i+1), so the exp latency is always covered by real PE work and the
    PE never waits on ScalarE.
  - out-projection PSUM->SBUF copies run on GpSimd, q/k/v copies on
    VectorE; output is written fp16 (host sums the 4 row-split partials
    in fp32).
  - matmuls in fp16 with fp32 PSUM accumulation.
  - one transient-NaN retry; numpy fallback for exotic masks/key padding
"""
import os
import sys

import numpy as np

for _p in ("/opt/trn_rl_repo", "/root/.axon_site/_ro/trn_rl_repo"):
    if os.path.isdir(_p) and _p not in sys.path:
        sys.path.insert(0, _p)
        break

import concourse.bacc as bacc
import concourse.mybir as mybir
import concourse.tile as tile
from concourse.bass_utils import run_bass_kernel_spmd

f32 = mybir.dt.float32
bf16 = mybir.dt.float16
f8 = mybir.dt.float8e4
AF = mybir.ActivationFunctionType
DR = mybir.MatmulPerfMode.DoubleRow

T, B, E, H, HD = 2048, 2, 1024, 16, 64
NCORES = 8
HL = (B * H) // NCORES          # heads per core = 4
J = HL * HD                     # per-core projection width = 256
EC = E // 128                   # e-chunks = 8
SCALE = HD ** -0.5
MCH = 512                       # m-chunk width
NEG_THRESH = -1e8               # "fully masked" threshold

SKIP, ZERO, ADD, ADDBIN = 0, 1, 2, 3

_prog_cache = {}


def _classify_mask(mask):
    """Classify 128x128 blocks of mask[t_query, s_key]."""
    nb = mask.shape[0] // 128
    blocks = mask.reshape(nb, 128, nb, 128)
    all_skip = (blocks <= NEG_THRESH).all(axis=(1, 3))
    all_zero = (blocks == 0.0).all(axis=(1, 3))
    binary = ((blocks == 0.0) | (blocks <= NEG_THRESH)).all(axis=(1, 3))
    cls = np.where(all_skip, SKIP,
                   np.where(all_zero, ZERO, np.where(binary, ADDBIN, ADD)))
    return cls  # [m_block, s_block]


def _build(T_, cls_key):
    cls = np.array(cls_key, dtype=np.int64)
    NB = T_ // 128
    NMC = T_ // MCH
    add_blocks = [(mb, sb) for mb in range(NB) for sb in range(NB)
                  if cls[mb, sb] == ADD]
    add_pos = {blk: i for i, blk in enumerate(add_blocks)}
    n_add = len(add_blocks)
    bin_blocks = [(mb, sb) for mb in range(NB) for sb in range(NB)
                  if cls[mb, sb] == ADDBIN]
    bin_pos = {blk: i for i, blk in enumerate(bin_blocks)}
    n_bin = len(bin_blocks)

    nc = bacc.Bacc("TRN2", target_bir_lowering=False, debug=False)
    xT = nc.declare_dram_parameter("xT", [E, T_], bf16, isOutput=False)
    wqpack = nc.declare_dram_parameter("wqpack", [128, EC * J], bf16,
                                       isOutput=False)
    wkvpack = nc.declare_dram_parameter("wkvpack", [128, 2 * EC * J], bf16,
                                        isOutput=False)
    wopack = nc.declare_dram_parameter("wopack", [128, (J // 128) * E], bf16,
                                       isOutput=False)
    bqp = nc.declare_dram_parameter("bqp", [128, 2], f32, isOutput=False)
    ones1 = nc.declare_dram_parameter("ones1", [1, 64], bf16, isOutput=False)
    msk = nc.declare_dram_parameter("msk", [128, max(n_add, 1) * 128], f32,
                                    isOutput=False)
    tri = nc.declare_dram_parameter("tri", [128, max(n_bin, 1) * 128], bf16,
                                    isOutput=False)
    out = nc.declare_dram_parameter("out", [T_, E], bf16, isOutput=True)

    # live (non-skip) trailing range per (chunk n, s-tile i): first live
    # 128-block within the chunk.  For causal masks the live blocks of a
    # chunk are a trailing contiguous range; fall back to ms=0 otherwise.
    def live_start(n, i):
        ks = [k for k in range(4) if cls[n * 4 + k, i] != SKIP]
        if not ks:
            return None
        k0 = min(ks)
        if ks != list(range(k0, 4)):
            return 0  # non-contiguous pattern: compute the full chunk
        return k0 * 128

    with tile.TileContext(nc) as tc:
        with nc.allow_low_precision(reason="bf16 matmuls, fp32 psum"), \
             tc.tile_pool(name="sba", bufs=1) as sba, \
             tc.tile_pool(name="sbw", bufs=1) as sbw, \
             tc.tile_pool(name="ps", bufs=1, space="PSUM") as ps:
            xT_cs = [sba.tile([128, T_], bf16, name=f"xT_c{c}")
                     for c in range(EC)]
            wpack_sb = sba.tile([128, 3 * EC * J], bf16)
            wq_sb = wpack_sb[:, 0:EC * J]
            wk_sb = wpack_sb[:, EC * J:2 * EC * J]
            wv_sb = wpack_sb[:, 2 * EC * J:3 * EC * J]
            wo_sb = sba.tile([128, (J // 128) * E], bf16)
            qT_sb = sba.tile([128, 2 * T_], bf16)
            kT_sb = sba.tile([128, 2 * T_], bf16)
            v_sb = sba.tile([128, HL * NB * 128], f8)
            oT_sb = sba.tile([128, 2 * T_], bf16)
            bq_sb = sba.tile([128, 2], f32)
            msk_sb = sba.tile([128, max(n_add, 1) * 128], f32)
            tri_sb = sba.tile([128, max(n_bin, 1) * 128], bf16)
            ones1_sb = sba.tile([1, 64], bf16)

            # ---- input DMAs: wq first, then xT chunk-by-chunk so the
            # first projection matmuls start as soon as chunk 0 lands ----
            nc.sync.dma_start(wq_sb, wqpack[:, :])
            nc.sync.dma_start(bq_sb[:], bqp[:, :])
            nc.sync.dma_start(ones1_sb[:], ones1[:, :])
            for c in range(EC):
                nc.sync.dma_start(xT_cs[c][:], xT[c * 128:(c + 1) * 128, :])
            nc.sync.dma_start(wpack_sb[:, EC * J:3 * EC * J], wkvpack[:, :])
            nc.sync.dma_start(wo_sb[:], wopack[:, :])
            # Each v strip is 128 wide: ones at col 0 (denominator lands
            # on PSUM partition 0 where the custom-DVE reciprocal needs
            # it), v at cols 64-127 (o lands on partitions 64-127, a
            # 64-aligned PSUM range for the DVE normalize multiply),
            # zeros in cols 1-63.
            v_view = v_sb[:].rearrange("p (x c) -> p x c", c=128)
            nc.vector.memset(v_view[:, :, 0:1], 1.0)
            nc.vector.memset(v_view[:, :, 1:64], 0.0)
            if n_add:
                nc.sync.dma_start(msk_sb[:], msk[:, :])
            if n_bin:
                nc.sync.dma_start(tri_sb[:], tri[:, :])

            # ================= projection building blocks ==============
            # Each "group" is one PSUM accumulation (8 chunk matmuls) +
            # one copy-out, cut into one-matmul thunks.  The PSUM tile is
            # allocated lazily inside the FIRST thunk so slot-rotation
            # order always matches instruction emission order.
            TB = {"pss": 2, "attno": 3, "big": 1}

            def qk_units(nn, u, wsb, dst, biased, tag):
                cell = []

                def mm(c):
                    def go():
                        if not cell:
                            cell.append(ps.tile([128, 512], f32, tag=tag,
                                                bufs=TB[tag], name="psq"))
                        nc.tensor.matmul(
                            cell[0][:],
                            wsb[:, c * J + u * 128: c * J + (u + 1) * 128],
                            xT_cs[c][:, nn * 512: nn * 512 + 512],
                            start=(c == 0), stop=(c == EC - 1),
                            skip_group_check=True)
                    return go

                def cp():
                    dslc = dst[:, u * T_ + nn * 512: u * T_ + nn * 512 + 512]
                    if biased:
                        nc.vector.tensor_scalar_add(dslc, cell[0][:],
                                                    bq_sb[:, u:u + 1])
                    else:
                        nc.vector.tensor_copy(dslc, cell[0][:])
                return [(mm(c), "mm") for c in range(EC)] + [(cp, "cp")]

            def v_units(i, tag):
                cell = []

                def mm(c):
                    def go():
                        if not cell:
                            cell.append(ps.tile([128, 512], f32, tag=tag,
                                                bufs=TB[tag], name="psv"))
                        nc.tensor.matmul(
                            cell[0][:, 0:J],
                            xT_cs[c][:, i * 128: i * 128 + 128],
                            wv_sb[:, c * J:(c + 1) * J],
                            start=(c == 0), stop=(c == EC - 1),
                            skip_group_check=True)
                    return go

                def cp():
                    dstv = v_sb[:, i * (HL * 128): (i + 1) * (HL * 128)] \
                        .rearrange("p (h c) -> p h c", c=128)[:, :, 64:128]
                    nc.vector.tensor_copy(dstv, cell[0][:, 0:J])
                return [(mm(c), "mm") for c in range(EC)] + [(cp, "cp")]

            # ---- prologue: 6 concurrent accumulation groups round-robin
            # over the (not-yet-used) attention psum slots, chunk-gated.
            V_UP = min(12, NB)
            vtags = ["attno", "big"] + ["pss", "pss",
                                        "attno", "attno", "attno", "big"] * 2
            pro_groups = [qk_units(0, 0, wq_sb, qT_sb, True, "pss"),
                          qk_units(0, 0, wk_sb, kT_sb, False, "pss"),
                          qk_units(0, 1, wq_sb, qT_sb, True, "attno"),
                          qk_units(0, 1, wk_sb, kT_sb, False, "attno")]
            pro_groups += [v_units(i, vtags[i]) for i in range(V_UP)]
            active = [list(g) for g in pro_groups[:6]]
            pend = pro_groups[6:]
            while active:
                nxt = []
                for g in active:
                    for _ in range(2):
                        if g:
                            g.pop(0)[0]()
                    if g:
                        nxt.append(g)
                    elif pend:
                        nxt.append(list(pend.pop(0)))
                active = nxt

            # ---- attention-phase filler units -------------------------
            # Each entry is (need_n, thunk): the unit MUST be emitted
            # before attention pair (need_n, 0) starts (emission-order
            # dependency), and is otherwise popped as PE filler.
            from collections import deque
            fill = deque()
            for nn in range(1, NMC):
                for u in range(2):
                    for wsb, dst, biased in ((wq_sb, qT_sb, True),
                                             (wk_sb, kT_sb, False)):
                        for th, kd in qk_units(nn, u, wsb, dst, biased, "big"):
                            fill.append((nn, th, kd))
                for i in range(4 * nn, min(4 * nn + 4, NB)):
                    if i >= V_UP:
                        for th, kd in v_units(i, "big"):
                            fill.append((nn, th, kd))
            for i in range(max(4 * NMC, V_UP), NB):
                for th, kd in v_units(i, "big"):
                    fill.append((NMC - 1, th, kd))

            def pop_fill(k):
                # pop up to k units; stop after a group-final copy so the
                # next group's first matmul (same PSUM slot) is emitted a
                # tile later, giving the copy time to drain (no PE stall)
                n = 0
                while fill and n < k:
                    _, th, kd = fill.popleft()
                    th()
                    n += 1
                    if kd == "cp":
                        break
                return n

            # ---- out-projection groups (become filler as oT completes)
            def out_proj_group(m16, eh, tag="big"):
                pso = ps.tile([128, 512], f32, tag=tag, bufs=TB[tag],
                              name="pso_op")
                for jc in range(J // 128):
                    nc.tensor.matmul(
                        pso[:],
                        oT_sb[:, jc * T_ + m16 * 128: jc * T_ + m16 * 128 + 128],
                        wo_sb[:, jc * E + eh * 512: jc * E + eh * 512 + 512],
                        start=(jc == 0), stop=(jc == J // 128 - 1),
                        skip_group_check=True)
                ob = sbw.tile([128, 512], bf16, tag="ob", bufs=4)
                if (m16 + eh) % 2 == 1:
                    nc.vector.tensor_copy(ob[:], pso[:])
                else:
                    nc.scalar.copy(ob[:], pso[:])
                nc.sync.dma_start(
                    out[m16 * 128:(m16 + 1) * 128,
                        eh * 512:(eh + 1) * 512], ob[:])

            def out_proj_thunks(n, tail=False, lo=0, hi=None):
                # tail groups rotate over the now-idle pss slots as well,
                # so back-to-back groups don't serialize on one PSUM bank
                def grp(m16, eh, tag):
                    return lambda: out_proj_group(m16, eh, tag)
                tags = ["pss", "big"] if tail else ["big"]
                pairs = [(m16, eh) for m16 in range(n * 4, n * 4 + 4)
                         for eh in range(E // 512)]
                return [grp(m16, eh, tags[k % len(tags)])
                        for k, (m16, eh) in enumerate(pairs)][lo:hi]

            # ---- normalize: fast reciprocal + partition-broadcast ----
            def normalize_units(pn, pu, pA, pB):
                """Units freeing psoA then psoB of pair (pn, pu).  No PE
                work at all: DVE reciprocal -> Pool partition_broadcast
                -> DVE multiply (PSUM x SBUF -> SBUF)."""
                recfs = [sbw.tile([1, 512], f32, tag="recf", bufs=4,
                                  name=f"recf{r}") for r in range(2)]

                def recip_half(pso_, r):
                    def go():
                        nc.vector.reciprocal_approx_fast(
                            recfs[r][0:1, 0:512], pso_[0:1, 0:512])
                    return go

                def norm_half(pso_, r, h):
                    u, poff = h >> 1, (h & 1) * 64

                    def go():
                        rb = sbw.tile([64, 512], f32, tag="rb", bufs=2)
                        nc.gpsimd.partition_broadcast(
                            rb[:], recfs[r][0:1, 0:512], channels=64)
                        nc.vector.tensor_mul(
                            oT_sb[poff:poff + 64,
                                  u * T_ + pn * 512: u * T_ + pn * 512 + 512],
                            pso_[64:128, :], rb[:])
                    return go

                return [recip_half(pA, 0), norm_half(pA, 0, 2 * pu),
                        recip_half(pB, 1), norm_half(pB, 1, 2 * pu + 1)]

            # ================= attention main loop =====================
            vv = v_sb[:].rearrange("p (i hc) -> p i hc", hc=HL * 128)

            def s_loop_pair(n, u, work):
                """work: deque of priority units (normalize/out-proj) to
                interleave; falls back to `fill` units.  AV is computed
                over PAIRS of consecutive s-tiles with an fp8 DoubleRow
                matmul (K=256, 0.5 cyc/col), deferred one group so exp
                latency is always covered by real PE work."""
                hA, hB = 2 * u, 2 * u + 1
                stiles = [i for i in range(NB) if live_start(n, i) is not None]
                psoA = ps.tile([128, 512], f32, tag="attno", bufs=3,
                               name="psoA")
                psoB = ps.tile([128, 512], f32, tag="attno", bufs=3,
                               name="psoB")
                qbase = u * T_ + n * 512
                # group consecutive s-tiles into DoubleRow pairs
                groups = []
                j = 0
                while j < len(stiles):
                    if (j + 1 < len(stiles)
                            and stiles[j + 1] == stiles[j] + 1):
                        groups.append((j, j + 1))
                        j += 2
                    else:
                        groups.append((j, None))
                        j += 1
                grp_of, plane_of = {}, {}
                for g, (a, b) in enumerate(groups):
                    grp_of[a] = g
                    plane_of[a] = 0
                    if b is not None:
                        grp_of[b] = g
                        plane_of[b] = 1
                ngr = len(groups)

                def do_av(g, ptp, info):
                    first, lastg = (g == 0), (g == ngr - 1)
                    (i0, ms0), p1 = info
                    for pso_, h, off in ((psoA, hA, 0), (psoB, hB, 512)):
                        strip0 = vv[:, i0, h * 128:(h + 1) * 128]
                        if p1 is None:
                            nc.tensor.matmul(
                                pso_[0:128, ms0:512], strip0,
                                ptp[:, off + ms0: off + 512],
                                start=first, stop=lastg,
                                skip_group_check=True)
                            continue
                        i1, ms1 = p1
                        if ms1 > ms0:
                            # leading columns only tile i0 reaches
                            nc.tensor.matmul(
                                pso_[0:128, ms0:ms1], strip0,
                                ptp[:, off + ms0: off + ms1],
                                start=first, stop=False,
                                skip_group_check=True)
                        lhsT = vv[:, i0:i0 + 2, h * 128:(h + 1) * 128]
                        rhs = ptp[:].rearrange(
                            "p (t hm) -> p t hm",
                            hm=1024)[:, :, off + ms1: off + 512]
                        nc.tensor.matmul(pso_[0:128, ms1:512], lhsT, rhs,
                                         start=first, stop=lastg,
                                         perf_mode=DR,
                                         skip_group_check=True)

                prev = None  # deferred AV: (group, ptp, info)
                ptp = None
                ginfo = None
                for idx, i in enumerate(stiles):
                    ms = live_start(n, i)
                    zero_ms = ms if idx == 0 else 0
                    if idx == 0:
                        ms = 0  # first tile zero-fills the full AV range
                    g = grp_of[idx]
                    plane = plane_of[idx]
                    if plane == 0:
                        ptp = sbw.tile([128, 2048], f8, tag="pt", bufs=3,
                                       name="ptp")
                        ginfo = [(i, ms), None]
                    else:
                        ginfo[1] = (i, ms)
                    pss = ps.tile([128, 1024], f32, tag="pss", bufs=2,
                                  name="pss")
                    kA = kT_sb[0:64, u * T_ + i * 128: u * T_ + i * 128 + 128]
                    kB = kT_sb[64:128, u * T_ + i * 128: u * T_ + i * 128 + 128]
                    qA = qT_sb[0:64, qbase + ms: qbase + 512]
                    qB = qT_sb[64:128, qbase + ms: qbase + 512]
                    nc.tensor.matmul(pss[:, ms:512], kA, qA,
                                     start=True, stop=True,
                                     skip_group_check=True)
                    nc.tensor.matmul(pss[:, 512 + ms:1024], kB, qB,
                                     start=True, stop=True,
                                     skip_group_check=True)
                    for k in range(4):
                        if cls[n * 4 + k, i] == ADD:
                            pos = add_pos[(n * 4 + k, i)]
                            mblk = msk_sb[:, pos * 128:(pos + 1) * 128]
                            for off in (0, 512):
                                nc.vector.tensor_add(
                                    pss[:, off + k * 128: off + (k + 1) * 128],
                                    pss[:, off + k * 128: off + (k + 1) * 128],
                                    mblk)
                    # one exp covering the live columns of both heads,
                    # written as fp8 into this group's plane
                    pb = plane * 1024
                    pv = pss[:].rearrange("p (h m) -> p h m", m=512)[:, :, ms:512]
                    tv = ptp[:, pb:pb + 1024].rearrange(
                        "p (h m) -> p h m", m=512)[:, :, ms:512]
                    nc.scalar.activation(tv, pv, AF.Exp)
                    if zero_ms:
                        # non-causal first tile: zero leading skip blocks
                        zv = ptp[:, pb:pb + 1024].rearrange(
                            "p (h m) -> p h m", m=512)[:, :, 0:zero_ms]
                        nc.vector.memset(zv, 0.0)
                    # filler between this tile's exp and the previous
                    # group's AV keeps the PE fed while ScalarE works
                    pop_fill(1)
                    if work:
                        work.popleft()()
                    else:
                        pop_fill(1)
                    if idx + 1 >= len(stiles) or grp_of[idx + 1] != g:
                        # group complete: emit the PREVIOUS group's AV
                        if prev is not None:
                            do_av(*prev)
                        prev = (g, ptp, ginfo)
                if work:
                    work.popleft()()
                else:
                    pop_fill(1)
                do_av(*prev)
                while work:
                    work.popleft()()
                    pop_fill(1)
                return psoA, psoB

            prevpair = None
            carry = deque()
            held = []
            for n in range(NMC):
                for u in range(2):
                    # mandatory: emit every fill unit pair (n, u) depends on
                    while fill and fill[0][0] <= n:
                        fill.popleft()[1]()
                    work = deque()
                    if prevpair is not None:
                        pn, pu, pA, pB = prevpair
                        work.extend(normalize_units(pn, pu, pA, pB))
                    work.extend(carry)
                    carry = deque()
                    psoA, psoB = s_loop_pair(n, u, work)
                    if prevpair is not None and pu == 1:
                        if pn == NMC - 2:
                            # hold back half of the second-to-last chunk's
                            # out-projection to overlap the final
                            # normalize chain after the last AV
                            carry.extend(out_proj_thunks(pn, lo=0, hi=4))
                            held = out_proj_thunks(pn, tail=True, lo=4)
                        else:
                            carry.extend(out_proj_thunks(pn))
                    prevpair = (n, u, psoA, psoB)
            pn, pu, pA, pB = prevpair
            tail = deque(normalize_units(pn, pu, pA, pB))
            held = deque(held if NMC >= 2 else [])
            # interleave the last pair's normalize (DVE/Pool) with the
            # held-back out-projection groups (PE work)
            while tail or held or carry:
                if carry:
                    carry.popleft()()
                if tail:
                    tail.popleft()()
                if held:
                    held.popleft()()
            while fill:
                fill.popleft()[1]()
            for w in out_proj_thunks(pn, tail=True):
                w()

    nc.compile()
    return nc


def _get_program(T_, cls):
    key = (T_, tuple(map(tuple, cls.tolist())))
    if key not in _prog_cache:
        _prog_cache[key] = _build(T_, key[1])
    return _prog_cache[key]


def _numpy_ref(query, attn_mask, key_padding_mask, Wq, bq, Wk, bk, Wv, bv,
               Wo, bo):
    """Exact-semantics fallback (mirrors reference.py in numpy)."""
    q = (query @ Wq.T + bq) * SCALE
    k = query @ Wk.T + bk
    v = query @ Wv.T + bv

    def shp(x):
        return x.reshape(T, B * H, HD).transpose(1, 0, 2)

    q, k, v = shp(q), shp(k), shp(v)
    w = np.einsum('bth,bsh->bts', q, k).reshape(B, H, T, T) + attn_mask
    w = np.where(key_padding_mask[:, None, None, :], -np.inf, w)
    w = w - w.max(axis=-1, keepdims=True)
    ew = np.exp(w)
    p = (ew / ew.sum(axis=-1, keepdims=True)).reshape(B * H, T, T)
    o = np.einsum('bts,bsh->bth', p, v.reshape(B * H, T, HD))
    o = o.transpose(1, 0, 2).reshape(T, B, E)
    return (o @ Wo.T + bo).astype(np.float32)


def _prep_inputs(query, attn_mask, Wq, bq, Wk, Wv, Wo, cls):
    """Build the 8 per-core input maps."""
    bf = np.float16
    add_blocks = [(mb, sb) for mb in range(T // 128) for sb in range(T // 128)
                  if cls[mb, sb] == ADD]
    n_add = len(add_blocks)
    if n_add:
        mskp = np.empty((128, n_add * 128), np.float32)
        for i, (mb, sb) in enumerate(add_blocks):
            blk = attn_mask[mb * 128:(mb + 1) * 128, sb * 128:(sb + 1) * 128]
            mskp[:, i * 128:(i + 1) * 128] = np.ascontiguousarray(blk.T)
    else:
        mskp = np.zeros((128, 128), np.float32)
    bin_blocks = [(mb, sb) for mb in range(T // 128) for sb in range(T // 128)
                  if cls[mb, sb] == ADDBIN]
    if bin_blocks:
        trip = np.empty((128, len(bin_blocks) * 128), bf)
        for i, (mb, sb) in enumerate(bin_blocks):
            blk = attn_mask[mb * 128:(mb + 1) * 128, sb * 128:(sb + 1) * 128]
            trip[:, i * 128:(i + 1) * 128] = (blk.T == 0.0).astype(bf)
    else:
        trip = np.zeros((128, 128), bf)
    ones1 = np.ones((1, 64), bf)

    in_maps = []
    for core in range(NCORES):
        b = core // (NCORES // B)
        jsl = slice((core % (NCORES // B)) * J, (core % (NCORES // B)) * J + J)
        EC_, J_ = E // 128, J

        def sb_layout(wT):  # [E, J] -> SBUF [128, EC*J]
            return np.ascontiguousarray(
                wT.reshape(EC_, 128, J_).transpose(1, 0, 2).reshape(128, EC_ * J_))

        xT_c = np.ascontiguousarray(query[:, b, :].T).astype(bf)
        wq_l = sb_layout((Wq[jsl, :] * np.float32(SCALE)).T)
        wk_l = sb_layout(Wk[jsl, :].T)
        wv_l = sb_layout(Wv[jsl, :].T)
        wqpack = np.ascontiguousarray(wq_l).astype(bf)
        wkvpack = np.concatenate([wk_l, wv_l], axis=1).astype(bf)
        woT = Wo[:, jsl].T  # [J, E]
        wopack = np.ascontiguousarray(
            woT.reshape(J_ // 128, 128, E).transpose(1, 0, 2)
            .reshape(128, (J_ // 128) * E)).astype(bf)
        bq_c = np.ascontiguousarray(
            (bq[jsl] * np.float32(SCALE)).reshape(2, 128).T)
        in_maps.append({
            "xT": xT_c, "wqpack": wqpack, "wkvpack": wkvpack,
            "wopack": wopack, "bqp": bq_c, "ones1": ones1, "msk": mskp,
            "tri": trip,
        })
    return in_maps


def _kernel_impl(inputs, trace=False, **run_kwargs):
    query = np.asarray(inputs["query"], np.float32)
    attn_mask = np.asarray(inputs["attn_mask"], np.float32)
    kpm = np.asarray(inputs["key_padding_mask"])
    Wq = np.asarray(inputs["Wq"], np.float32)
    bq = np.asarray(inputs["bq"], np.float32)
    Wk = np.asarray(inputs["Wk"], np.float32)
    bk = np.asarray(inputs["bk"], np.float32)
    Wv = np.asarray(inputs["Wv"], np.float32)
    bv = np.asarray(inputs["bv"], np.float32)
    Wo = np.asarray(inputs["Wo"], np.float32)
    bo = np.asarray(inputs["bo"], np.float32)

    # Fast path requires: no key padding, no fully-masked rows, block-
    # classifiable mask with a modest number of additive blocks, and no
    # bk dependence issue (bk shifts are softmax-invariant, always ok).
    cls = _classify_mask(attn_mask)
    fallback = (
        kpm.any()
        or (attn_mask.max(axis=1) <= NEG_THRESH).any()
        or (cls == ADD).sum() > 24 or (cls == ADDBIN).sum() > 24
        or np.isnan(attn_mask).any()
    )
    if fallback:
        return _numpy_ref(query, attn_mask, kpm, Wq, bq, Wk, bk, Wv, bv,
                          Wo, bo), None

    nc = _get_program(T, cls)
    in_maps = _prep_inputs(query, attn_mask, Wq, bq, Wk, Wv, Wo, cls)
    for attempt in range(3):
        res = run_bass_kernel_spmd(nc, in_maps, core_ids=list(range(NCORES)),
                                   trace=trace, **run_kwargs)
        if all(np.isfinite(r["out"]).all() for r in res.results):
            break
    else:
        return _numpy_ref(query, attn_mask, kpm, Wq, bq, Wk, bk, Wv, bv,
                          Wo, bo), None

    # unshard: sum the 4 row-split partials per batch element (the Wo
    # all-reduce), then add bo and the bv contribution (sum_s p = 1).
    bo_total = bo + Wo @ bv
    out = np.empty((T, B, E), np.float32)
    gsz = NCORES // B
    for b in range(B):
        acc = res.results[b * gsz]["out"].astype(np.float32)
        for c in range(b * gsz + 1, (b + 1) * gsz):
            acc = acc + res.results[c]["out"].astype(np.float32)
        out[:, b, :] = acc + bo_total[None, :]
    return out, res


def kernel(**inputs):
    out, _ = _kernel_impl(inputs, trace=False)
    return out
